# revision 37
# baseline (speedup 1.0000x reference)
"""Self-contained Trainium2 Bass kernel for nn_AttentionPooling.

Contract: kernel(**inputs) takes FULL unsharded numpy inputs (as produced by
setup_inputs) and returns the FULL output tuple (logits [128,16,1],
attn_weights [128,1,16,512]), both float32.

Strategy
--------
Data-parallel over batch: 8 NeuronCores x 16 batch elements each, processed
in PAIRS to halve per-instruction overhead (a [64,512] vector op costs the
same as a [16,512] one — cost scales with the free dimension only).

Math restructuring (exact, validated vs the reference to ~3e-6 rel):
Because H == 1 and the query is batch-independent, the Q/K projections
collapse into one effective [16, 512] matrix G' = ((LN(query*scale) @ Wq.T
+ bq) @ Wk) * ln_lat_w, and the V/output projections collapse into a single
effective vector wvdev = scale * (Wo @ Wv) * ln_lat_w.  The latent-side
LayerNorm folds algebraically into per-column statistics (mean and rstd of
x = scale * lat') computed with extra matmul rows:

  lat'      = latents + PE^T / scale           (host, exact; then bf16)
  A2[q,n]   = sum_d gdev2[q,d] lat'[d,n]       (PE; gdev2 = G' - sG/512
                                                folds the mean-centering)
  slat[n]   = sum_d lat'[d,n]                  (PE, lhsT col 16 = ones)
  svraw2[n] = sum_d wvdev2[d] lat'[d,n]        (PE, col 17; wvdev2 centered)
  su[n]     = sum_d lat'[d,n]^2                (PE over u = lat'*lat')
  m~ = slat/512,  var = scale^2 (su/512 - m~^2),  r = 1/sqrt(var + eps)
  S[q,n]    = qm01[q] * r[n] * A2[q,n]
  attn_w    = softmax_n(S)      (no max-subtraction: |S| <~ 2 for this data)
  vWo[n]    = r[n] * svraw2[n]
  logits[q] = qm01[q] * (sum_n e[q,n] vWo[n] / sum_n e[q,n] + c_v) + bo

Row-constant score shifts are dropped (softmax-invariant), and the
query_mask path is exact: a zero mask row zeroes S (uniform softmax,
matching the reference's -1e9 fill) and routes logits to bo.

Per-pair PSUM layout (one bank, partitions):
  [ 0:32)  batch b0: A2 rows 0-15, slat 16, svraw2 17, zeros 18-31
  [32:64)  batch b1: same at +32
  [64:96)  su accumulators: row 64 = su(b0), row 65 = su(b1), rest zeros
Phase-2 ops run on the full [0:64) range with "junk lanes" masked by a
zero query-mask column — same instruction cost, half the instruction count.
"""

import math
from contextlib import ExitStack

import numpy as np
import ml_dtypes

import concourse.bass as bass
import concourse.mybir as mybir
import concourse.tile as tile
from concourse import bacc
from concourse.bass_utils import run_bass_kernel_spmd

F32 = mybir.dt.float32
BF16 = mybir.dt.bfloat16
Alu = mybir.AluOpType
Act = mybir.ActivationFunctionType

D = 512          # d_model
N = 512          # sequence length
NQ = 16          # number of queries
B = 128          # batch
NCORES = 8
PB = B // NCORES  # batches per core = 16
NPAIR = PB // 2
SCALE = math.sqrt(D)
EPS = 1e-5
KC = 4            # contraction chunks (512 / 128)
GROUP = 16       # batches per stats group (single group: dense PE burst)
PAIRS_PER_GROUP = GROUP // 2


def _pos_encoding_T(d_model: int, seq_len: int) -> np.ndarray:
    """Reversed sinusoidal PE, transposed to [D, N] (float64)."""
    pos = np.arange(seq_len, dtype=np.float64)[:, None]
    i = np.arange(0, d_model, 2, dtype=np.float64)
    ang = pos * np.exp(i * (-(math.log(10000.0) / d_model)))  # [N, d/2]
    pe = np.stack([np.sin(ang), np.cos(ang)], axis=-1).reshape(seq_len, d_model)
    return pe[::-1].T.copy()  # [D, N]


def _layernorm_np(x, w, b, eps=1e-5):
    m = x.mean(-1, keepdims=True)
    v = ((x - m) ** 2).mean(-1, keepdims=True)
    return (x - m) / np.sqrt(v + eps) * w + b


def _build_kernel():
    """Build the per-core Bass program (identical across cores)."""
    nc = bacc.Bacc("TRN2", target_bir_lowering=False, debug=False)

    d_latp = nc.dram_tensor("latp", [PB, 128, KC, N], BF16, kind="ExternalInput")
    d_lhsT = nc.dram_tensor("lhsT", [128, KC, 32], BF16, kind="ExternalInput")
    # u-matmul weights: [:, 0, :] has ones in col 0 (b0), [:, 1, :] ones in
    # col 1 (b1); all other cols zero.
    d_uw = nc.dram_tensor("uw", [128, 2, 32], BF16, kind="ExternalInput")
    d_qm = nc.dram_tensor("qm", [64, NPAIR], F32, kind="ExternalInput")
    # cc columns: 0 = unused, 1 = c_v, 2 = bo, 3 = eps
    d_cc = nc.dram_tensor("cc", [64, 4], F32, kind="ExternalInput")
    d_aw = nc.dram_tensor("aw", [PB, NQ, N], F32, kind="ExternalOutput")
    d_lg = nc.dram_tensor("lg", [64, NPAIR], F32, kind="ExternalOutput")

    with tile.TileContext(nc) as tc, ExitStack() as ctx:
        const = ctx.enter_context(tc.tile_pool(name="const", bufs=1))
        gstats = ctx.enter_context(tc.tile_pool(name="gstats", bufs=2))
        latp = ctx.enter_context(tc.tile_pool(name="latp", bufs=5))
        up = ctx.enter_context(tc.tile_pool(name="up", bufs=3))
        psum = ctx.enter_context(tc.tile_pool(name="psum", bufs=6, space="PSUM"))
        apool = ctx.enter_context(tc.tile_pool(name="apool", bufs=PAIRS_PER_GROUP + 2))
        work = ctx.enter_context(tc.tile_pool(name="work", bufs=4))
        small = ctx.enter_context(tc.tile_pool(name="small", bufs=4))
        dram = ctx.enter_context(tc.tile_pool(name="dram", bufs=1, space="DRAM"))
        r_dram = dram.tile([PB, N], F32, tag="r_dram")
        v_dram = dram.tile([PB, N], F32, tag="v_dram")

        lhsT_sb = const.tile([128, KC, 32], BF16, tag="lhsT")
        uw_sb = const.tile([128, 2, 32], BF16, tag="uw")
        qm_sb = const.tile([64, NPAIR], F32, tag="qm")
        cc_sb = const.tile([64, 4], F32, tag="cc")
        nc.sync.dma_start(out=lhsT_sb[:], in_=d_lhsT.ap())
        nc.sync.dma_start(out=uw_sb[:], in_=d_uw.ap())
        nc.sync.dma_start(out=qm_sb[:], in_=d_qm.ap())
        nc.sync.dma_start(out=cc_sb[:], in_=d_cc.ap())
        logits_all = const.tile([64, NPAIR], F32, tag="lg")

        c_inv_d = 1.0 / D
        c_var = (SCALE * SCALE) / D

        for g0 in range(0, PB, GROUP):
            # Per-group stats tiles (compute ops need 32-aligned partition
            # bases, so stacked rows live in group-local tiles at base 0).
            rows_g = gstats.tile([GROUP, 3, N], F32, tag="rows")
            slat_g = rows_g[:, 0, :]
            svraw_g = rows_g[:, 1, :]
            su_g = rows_g[:, 2, :]
            m_g = gstats.tile([GROUP, N], F32, tag="m")
            m2s_g = gstats.tile([GROUP, N], F32, tag="m2s")
            var_g = gstats.tile([GROUP, N], F32, tag="var")
            rstd_g = gstats.tile([GROUP, N], F32, tag="rstd")
            r_g = gstats.tile([GROUP, N], F32, tag="r")
            vwo_g = gstats.tile([GROUP, N], F32, tag="vwo")
            a_tiles = {}
            for j in range(g0 // 2, g0 // 2 + PAIRS_PER_GROUP):
                b0 = 2 * j
                i0 = b0 - g0  # row offset within the group tiles
                lat_p = latp.tile([128, 2, KC, N], BF16, tag="lat")
                nc.sync.dma_start(
                    out=lat_p[:],
                    in_=d_latp.ap()[b0 : b0 + 2].rearrange("b p c n -> p b c n"),
                )
                u_p = up.tile([128, 2, KC, N], BF16, tag="u")
                for i in range(2):
                    nc.vector.tensor_mul(
                        u_p[:, i].rearrange("p c n -> p (c n)"),
                        lat_p[:, i].rearrange("p c n -> p (c n)"),
                        lat_p[:, i].rearrange("p c n -> p (c n)"),
                    )
                ps = psum.tile([128, N], F32, tag="ps")
                for i in range(2):
                    for c in range(KC):
                        nc.tensor.matmul(
                            ps[32 * i : 32 * i + 32, :], lhsT_sb[:, c, :],
                            lat_p[:, i, c, :],
                            start=(c == 0), stop=(c == KC - 1),
                        )
                for i in range(2):
                    for c in range(KC):
                        nc.tensor.matmul(
                            ps[64:96, :], uw_sb[:, i, :], u_p[:, i, c, :],
                            start=(i == 0 and c == 0),
                            stop=(i == 1 and c == KC - 1),
                        )
                a_p = apool.tile([66, N], F32, tag="A")
                nc.scalar.copy(a_p[:], ps[0:66, :])
                # Gather stats rows (slat @16/48, svraw @17/49, su @64/65)
                # into per-batch slots of rows_g: [batch, {slat,svraw,su}, N].
                nc.scalar.dma_start(
                    out=rows_g[i0 : i0 + 1, 0:2, :], in_=a_p[16:18, :]
                )
                nc.scalar.dma_start(
                    out=rows_g[i0 + 1 : i0 + 2, 0:2, :], in_=a_p[48:50, :]
                )
                nc.scalar.dma_start(
                    out=rows_g[i0 : i0 + 2, 2:3, :], in_=a_p[64:66, :]
                )
                a_tiles[j] = a_p

            # Batched LN statistics for the group (per-column mean / rstd).
            nc.vector.scalar_tensor_tensor(
                out=m2s_g[:], in0=slat_g, scalar=(SCALE * SCALE) / (D * D),
                in1=slat_g, op0=Alu.mult, op1=Alu.mult,
            )
            nc.vector.scalar_tensor_tensor(
                out=var_g[:], in0=su_g, scalar=c_var,
                in1=m2s_g[:], op0=Alu.mult, op1=Alu.subtract,
            )
            nc.scalar.activation(
                out=rstd_g[:], in_=var_g[:], func=Act.Sqrt,
                bias=cc_sb[0:GROUP, 3:4], scale=1.0,
            )
            nc.vector.reciprocal_approx_fast(out=r_g[:], in_=rstd_g[:])
            nc.vector.tensor_mul(vwo_g[:], svraw_g, r_g[:])
            gsl = slice(g0, g0 + GROUP)
            nc.sync.dma_start(out=r_dram[gsl, :], in_=r_g[:])
            nc.sync.dma_start(out=v_dram[gsl, :], in_=vwo_g[:])

            for j in range(g0 // 2, g0 // 2 + PAIRS_PER_GROUP):
                b0 = 2 * j
                a_p = a_tiles[j]
                rb = work.tile([64, N], F32, tag="rb")
                vb = work.tile([64, N], F32, tag="vb")
                nc.scalar.dma_start(
                    out=rb[:], in_=r_dram[b0 : b0 + 2, :].rearrange("b (x n) -> b x n", x=1).broadcast_to((2, 32, N))
                )
                nc.scalar.dma_start(
                    out=vb[:], in_=v_dram[b0 : b0 + 2, :].rearrange("b (x n) -> b x n", x=1).broadcast_to((2, 32, N))
                )
                s2 = work.tile([64, N], F32, tag="s2")
                nc.vector.scalar_tensor_tensor(
                    out=s2[:], in0=a_p[0:64, :], scalar=qm_sb[:, j : j + 1],
                    in1=rb[:], op0=Alu.mult, op1=Alu.mult,
                )
                e = work.tile([64, N], F32, tag="e")
                rowsum = small.tile([64, 1], F32, tag="rowsum")
                nc.scalar.activation(
                    out=e[:], in_=s2[:], func=Act.Exp, accum_out=rowsum[:]
                )
                pw = work.tile([64, N], F32, tag="pw")
                nc.gpsimd.normalize_recip(out_ap=pw[:], in_ap=e[:], denom_ap=rowsum[:])
                rinv = rowsum
                nc.scalar.dma_start(out=d_aw.ap()[b0], in_=pw[0:NQ, :])
                nc.scalar.dma_start(out=d_aw.ap()[b0 + 1], in_=pw[32 : 32 + NQ, :])
                ev = work.tile([64, N], F32, tag="ev")
                nume = small.tile([64, 1], F32, tag="nume")
                nc.vector.scalar_tensor_tensor(
                    out=ev[:], in0=e[:], scalar=1.0, in1=vb[:],
                    op0=Alu.mult, op1=Alu.mult, accum_out=nume[:],
                )
                l1 = small.tile([64, 1], F32, tag="l1")
                nc.vector.scalar_tensor_tensor(
                    out=l1[:], in0=nume[:], scalar=rinv[:],
                    in1=cc_sb[:, 1:2], op0=Alu.mult, op1=Alu.add,
                )
                nc.vector.scalar_tensor_tensor(
                    out=logits_all[:, j : j + 1], in0=l1[:],
                    scalar=qm_sb[:, j : j + 1], in1=cc_sb[:, 2:3],
                    op0=Alu.mult, op1=Alu.add,
                )

        nc.sync.dma_start(out=d_lg.ap(), in_=logits_all[:])

    nc.finalize()
    return nc


def _host_prep(latents, query_mask, query, ln_lat_w, ln_lat_b, ln_q_w, ln_q_b,
               Wq, bq, Wk, bk, Wv, bv, Wo, bo):
    f64 = np.float64
    pet = _pos_encoding_T(D, N)  # [D, N] float64

    q = query.astype(f64) * SCALE
    q_ln = _layernorm_np(q, ln_q_w.astype(f64), ln_q_b.astype(f64))
    qh = q_ln @ Wq.astype(f64).T + bq.astype(f64)          # [16, D]
    G = qh @ Wk.astype(f64)                                 # [16, D]
    w = ln_lat_w.astype(f64)
    b_ = ln_lat_b.astype(f64)
    gdev = (SCALE / math.sqrt(D)) * (G * w)                 # == G * w (H=1)
    sG = gdev.sum(-1)                                       # [16]
    gdev2 = gdev - sG[:, None] / D                          # fold mean-centering
    wv_eff = (Wo.astype(f64) @ Wv.astype(f64))[0]           # [D]
    wvdev = SCALE * (wv_eff * w)                            # [D]
    sv = float(wvdev.sum())
    wvdev2 = wvdev - sv / D
    c_v = float(wv_eff @ b_ + Wo.astype(f64)[0] @ bv.astype(f64))
    bo_val = float(np.asarray(bo).reshape(-1)[0])

    # lat' = latents + PE^T/scale, bf16, laid out [PB, 128, KC, N] per core
    latp = latents.astype(np.float32) + (pet / SCALE).astype(np.float32)[None]
    latp = latp.astype(ml_dtypes.bfloat16)                  # [B, D, N]
    latp = latp.reshape(B, KC, 128, N).transpose(0, 2, 1, 3)  # [B, 128, KC, N]
    latp = np.ascontiguousarray(latp)

    # lhsT [128, KC, 32]: cols 0..15 = gdev2^T, 16 = ones, 17 = wvdev2, rest 0
    lhsT = np.zeros((KC, 128, 32), np.float32)
    gdT = gdev2.T.reshape(KC, 128, NQ)                      # [KC, 128, 16]
    lhsT[:, :, 0:NQ] = gdT
    lhsT[:, :, NQ] = 1.0
    lhsT[:, :, NQ + 1] = wvdev2.reshape(KC, 128)
    lhsT = np.ascontiguousarray(
        lhsT.transpose(1, 0, 2)).astype(ml_dtypes.bfloat16)  # [128, KC, 32]

    uw = np.zeros((128, 2, 32), np.float32)
    uw[:, 0, 0] = 1.0
    uw[:, 1, 1] = 1.0
    uw = uw.astype(ml_dtypes.bfloat16)

    qm01 = (query_mask.astype(np.float32) != 0).astype(np.float32)  # [B, 16]

    cc = np.zeros((64, 4), np.float32)
    cc[:, 1] = c_v
    cc[:, 2] = bo_val
    cc[:, 3] = EPS

    in_maps = []
    for core in range(NCORES):
        bsl = slice(core * PB, (core + 1) * PB)
        qmc = qm01[bsl]                                     # [PB, 16]
        qm64 = np.zeros((64, NPAIR), np.float32)
        for j in range(NPAIR):
            qm64[0:16, j] = qmc[2 * j]
            qm64[32:48, j] = qmc[2 * j + 1]
        in_maps.append({
            "latp": latp[bsl],
            "lhsT": lhsT,
            "uw": uw,
            "qm": qm64,
            "cc": cc,
        })
    return in_maps


def kernel(latents, query_mask, query, ln_lat_w, ln_lat_b, ln_q_w, ln_q_b,
           Wq, bq, Wk, bk, Wv, bv, Wo, bo):
    args = [np.asarray(a) for a in (
        latents, query_mask, query, ln_lat_w, ln_lat_b, ln_q_w, ln_q_b,
        Wq, bq, Wk, bk, Wv, bv, Wo, bo)]
    in_maps = _host_prep(*args)
    nc = _build_kernel()
    res = run_bass_kernel_spmd(nc, in_maps, core_ids=list(range(NCORES)))
    aw = np.concatenate([r["aw"] for r in res.results], axis=0)   # [B, 16, N]
    lgs = []
    for r in res.results:
        lg64 = r["lg"]                                       # [64, NPAIR]
        lgc = np.empty((PB, NQ), np.float32)
        for j in range(NPAIR):
            lgc[2 * j] = lg64[0:16, j]
            lgc[2 * j + 1] = lg64[32:48, j]
        lgs.append(lgc)
    logits = np.concatenate(lgs, axis=0)[:, :, None].astype(np.float32)
    attn_weights = aw[:, None, :, :].astype(np.float32)
    return logits, attn_weights


# revision 38
# speedup vs baseline: 1.1448x; 1.1448x over previous
"""Self-contained Trainium2 Bass kernel for nn_AttentionPooling.

Contract: kernel(**inputs) takes FULL unsharded numpy inputs (as produced by
setup_inputs) and returns the FULL output tuple (logits [128,16,1],
attn_weights [128,1,16,512]), both float32.

Strategy
--------
Data-parallel over batch: 8 NeuronCores x 16 batch elements each, processed
in PAIRS to halve per-instruction overhead (a [64,512] vector op costs the
same as a [16,512] one — cost scales with the free dimension only).

Math restructuring (exact, validated vs the reference to ~3e-6 rel):
Because H == 1 and the query is batch-independent, the Q/K projections
collapse into one effective [16, 512] matrix G' = ((LN(query*scale) @ Wq.T
+ bq) @ Wk) * ln_lat_w, and the V/output projections collapse into a single
effective vector wvdev = scale * (Wo @ Wv) * ln_lat_w.  The latent-side
LayerNorm folds algebraically into per-column statistics (mean and rstd of
x = scale * lat') computed with extra matmul rows:

  lat'      = latents + PE^T / scale           (host, exact; then bf16)
  A2[q,n]   = sum_d gdev2[q,d] lat'[d,n]       (PE; gdev2 = G' - sG/512
                                                folds the mean-centering)
  slat[n]   = sum_d lat'[d,n]                  (PE, lhsT col 16 = ones)
  svraw2[n] = sum_d wvdev2[d] lat'[d,n]        (PE, col 17; wvdev2 centered)
  su[n]     = sum_d lat'[d,n]^2                (PE over u = lat'*lat')
  m~ = slat/512,  var = scale^2 (su/512 - m~^2),  r = 1/sqrt(var + eps)
  S[q,n]    = qm01[q] * r[n] * A2[q,n]
  attn_w    = softmax_n(S)      (no max-subtraction: |S| <~ 2 for this data)
  vWo[n]    = r[n] * svraw2[n]
  logits[q] = qm01[q] * (sum_n e[q,n] vWo[n] / sum_n e[q,n] + c_v) + bo

Row-constant score shifts are dropped (softmax-invariant), and the
query_mask path is exact: a zero mask row zeroes S (uniform softmax,
matching the reference's -1e9 fill) and routes logits to bo.

Per-pair PSUM layout (one bank, partitions):
  [ 0:32)  batch b0: A2 rows 0-15, slat 16, svraw2 17, zeros 18-31
  [32:64)  batch b1: same at +32
  [64:96)  su accumulators: row 64 = su(b0), row 65 = su(b1), rest zeros
Phase-2 ops run on the full [0:64) range with "junk lanes" masked by a
zero query-mask column — same instruction cost, half the instruction count.
"""

import math
from contextlib import ExitStack

import numpy as np
import ml_dtypes

import concourse.bass as bass
import concourse.mybir as mybir
import concourse.tile as tile
from concourse import bacc
from concourse.bass_utils import run_bass_kernel_spmd

F32 = mybir.dt.float32
BF16 = mybir.dt.bfloat16
Alu = mybir.AluOpType
Act = mybir.ActivationFunctionType

D = 512          # d_model
N = 512          # sequence length
NQ = 16          # number of queries
B = 128          # batch
NCORES = 8
PB = B // NCORES  # batches per core = 16
NPAIR = PB // 2
SCALE = math.sqrt(D)
EPS = 1e-5
KC = 4            # contraction chunks (512 / 128)
GROUP = 16       # batches per stats group (single group: dense PE burst)
PAIRS_PER_GROUP = GROUP // 2


def _pos_encoding_T(d_model: int, seq_len: int) -> np.ndarray:
    """Reversed sinusoidal PE, transposed to [D, N] (float64)."""
    pos = np.arange(seq_len, dtype=np.float64)[:, None]
    i = np.arange(0, d_model, 2, dtype=np.float64)
    ang = pos * np.exp(i * (-(math.log(10000.0) / d_model)))  # [N, d/2]
    pe = np.stack([np.sin(ang), np.cos(ang)], axis=-1).reshape(seq_len, d_model)
    return pe[::-1].T.copy()  # [D, N]


def _layernorm_np(x, w, b, eps=1e-5):
    m = x.mean(-1, keepdims=True)
    v = ((x - m) ** 2).mean(-1, keepdims=True)
    return (x - m) / np.sqrt(v + eps) * w + b


def _build_kernel():
    """Build the per-core Bass program (identical across cores)."""
    nc = bacc.Bacc("TRN2", target_bir_lowering=False, debug=False)

    d_latp = nc.dram_tensor("latp", [PB, 128, KC, N], BF16, kind="ExternalInput")
    d_lhsT = nc.dram_tensor("lhsT", [128, KC, 32], BF16, kind="ExternalInput")
    # u-matmul weights: [:, 0, :] has ones in col 0 (b0), [:, 1, :] ones in
    # col 1 (b1); all other cols zero.
    d_uw = nc.dram_tensor("uw", [128, 2, 32], BF16, kind="ExternalInput")
    d_qm = nc.dram_tensor("qm", [64, NPAIR], F32, kind="ExternalInput")
    # cc columns: 0 = unused, 1 = c_v, 2 = bo, 3 = eps
    d_cc = nc.dram_tensor("cc", [64, 4], F32, kind="ExternalInput")
    d_aw = nc.dram_tensor("aw", [PB, NQ, N], F32, kind="ExternalOutput")
    d_lg = nc.dram_tensor("lg", [64, NPAIR], F32, kind="ExternalOutput")

    with tile.TileContext(nc) as tc, ExitStack() as ctx:
        const = ctx.enter_context(tc.tile_pool(name="const", bufs=1))
        gstats = ctx.enter_context(tc.tile_pool(name="gstats", bufs=2))
        latp = ctx.enter_context(tc.tile_pool(name="latp", bufs=5))
        up = ctx.enter_context(tc.tile_pool(name="up", bufs=3))
        psum = ctx.enter_context(tc.tile_pool(name="psum", bufs=6, space="PSUM"))
        apool = ctx.enter_context(tc.tile_pool(name="apool", bufs=PAIRS_PER_GROUP + 2))
        work = ctx.enter_context(tc.tile_pool(name="work", bufs=4))
        small = ctx.enter_context(tc.tile_pool(name="small", bufs=4))
        dram = ctx.enter_context(tc.tile_pool(name="dram", bufs=1, space="DRAM"))
        r_dram = dram.tile([PB, N], F32, tag="r_dram")
        v_dram = dram.tile([PB, N], F32, tag="v_dram")

        lhsT_sb = const.tile([128, KC, 32], BF16, tag="lhsT")
        uw_sb = const.tile([128, 2, 32], BF16, tag="uw")
        qm_sb = const.tile([64, NPAIR], F32, tag="qm")
        cc_sb = const.tile([64, 4], F32, tag="cc")
        nc.sync.dma_start(out=lhsT_sb[:], in_=d_lhsT.ap())
        nc.sync.dma_start(out=uw_sb[:], in_=d_uw.ap())
        nc.sync.dma_start(out=qm_sb[:], in_=d_qm.ap())
        nc.sync.dma_start(out=cc_sb[:], in_=d_cc.ap())
        logits_all = const.tile([64, NPAIR], F32, tag="lg")

        c_inv_d = 1.0 / D
        c_var = (SCALE * SCALE) / D

        for g0 in range(0, PB, GROUP):
            # Per-group stats tiles (compute ops need 32-aligned partition
            # bases, so stacked rows live in group-local tiles at base 0).
            rows_g = gstats.tile([GROUP, 3, N], F32, tag="rows")
            slat_g = rows_g[:, 0, :]
            svraw_g = rows_g[:, 1, :]
            su_g = rows_g[:, 2, :]
            m_g = gstats.tile([GROUP, N], F32, tag="m")
            m2s_g = gstats.tile([GROUP, N], F32, tag="m2s")
            var_g = gstats.tile([GROUP, N], F32, tag="var")
            rstd_g = gstats.tile([GROUP, N], F32, tag="rstd")
            r_g = gstats.tile([GROUP, N], F32, tag="r")
            vwo_g = gstats.tile([GROUP, N], F32, tag="vwo")
            a_tiles = {}
            for j in range(g0 // 2, g0 // 2 + PAIRS_PER_GROUP):
                b0 = 2 * j
                i0 = b0 - g0  # row offset within the group tiles
                lat_p = latp.tile([128, 2, KC, N], BF16, tag="lat")
                nc.sync.dma_start(
                    out=lat_p[:],
                    in_=d_latp.ap()[b0 : b0 + 2].rearrange("b p c n -> p b c n"),
                )
                u_p = up.tile([128, 2, KC, N], BF16, tag="u")
                for i in range(2):
                    nc.vector.tensor_mul(
                        u_p[:, i].rearrange("p c n -> p (c n)"),
                        lat_p[:, i].rearrange("p c n -> p (c n)"),
                        lat_p[:, i].rearrange("p c n -> p (c n)"),
                    )
                ps = psum.tile([128, N], F32, tag="ps")
                for i in range(2):
                    for c in range(KC):
                        nc.tensor.matmul(
                            ps[32 * i : 32 * i + 32, :], lhsT_sb[:, c, :],
                            lat_p[:, i, c, :],
                            start=(c == 0), stop=(c == KC - 1),
                        )
                for i in range(2):
                    for c in range(KC):
                        nc.tensor.matmul(
                            ps[64:96, :], uw_sb[:, i, :], u_p[:, i, c, :],
                            start=(i == 0 and c == 0),
                            stop=(i == 1 and c == KC - 1),
                        )
                a_p = apool.tile([66, N], F32, tag="A")
                nc.scalar.copy(a_p[:], ps[0:66, :])
                # Gather stats rows (slat @16/48, svraw @17/49, su @64/65)
                # into per-batch slots of rows_g: [batch, {slat,svraw,su}, N].
                nc.scalar.dma_start(
                    out=rows_g[i0 : i0 + 1, 0:2, :], in_=a_p[16:18, :]
                )
                nc.scalar.dma_start(
                    out=rows_g[i0 + 1 : i0 + 2, 0:2, :], in_=a_p[48:50, :]
                )
                nc.scalar.dma_start(
                    out=rows_g[i0 : i0 + 2, 2:3, :], in_=a_p[64:66, :]
                )
                a_tiles[j] = a_p

            # Batched LN statistics for the group (per-column mean / rstd).
            nc.vector.scalar_tensor_tensor(
                out=m2s_g[:], in0=slat_g, scalar=(SCALE * SCALE) / (D * D),
                in1=slat_g, op0=Alu.mult, op1=Alu.mult,
            )
            nc.vector.scalar_tensor_tensor(
                out=var_g[:], in0=su_g, scalar=c_var,
                in1=m2s_g[:], op0=Alu.mult, op1=Alu.subtract,
            )
            nc.scalar.activation(
                out=rstd_g[:], in_=var_g[:], func=Act.Sqrt,
                bias=cc_sb[0:GROUP, 3:4], scale=1.0,
            )
            nc.vector.reciprocal_approx_fast(out=r_g[:], in_=rstd_g[:])
            nc.vector.tensor_mul(vwo_g[:], svraw_g, r_g[:])
            gsl = slice(g0, g0 + GROUP)
            nc.sync.dma_start(out=r_dram[gsl, :], in_=r_g[:])
            nc.sync.dma_start(out=v_dram[gsl, :], in_=vwo_g[:])

            for j in range(g0 // 2, g0 // 2 + PAIRS_PER_GROUP):
                b0 = 2 * j
                a_p = a_tiles[j]
                rb = work.tile([64, N], F32, tag="rb")
                vb = work.tile([64, N], F32, tag="vb")
                nc.gpsimd.dma_start(
                    out=rb[:], in_=r_dram[b0 : b0 + 2, :].rearrange("b (x n) -> b x n", x=1).broadcast_to((2, 32, N))
                )
                nc.gpsimd.dma_start(
                    out=vb[:], in_=v_dram[b0 : b0 + 2, :].rearrange("b (x n) -> b x n", x=1).broadcast_to((2, 32, N))
                )
                s2 = work.tile([64, N], F32, tag="s2")
                nc.vector.scalar_tensor_tensor(
                    out=s2[:], in0=a_p[0:64, :], scalar=qm_sb[:, j : j + 1],
                    in1=rb[:], op0=Alu.mult, op1=Alu.mult,
                )
                e = work.tile([64, N], F32, tag="e")
                rowsum = small.tile([64, 1], F32, tag="rowsum")
                nc.scalar.activation(
                    out=e[:], in_=s2[:], func=Act.Exp, accum_out=rowsum[:]
                )
                pw = work.tile([64, N], F32, tag="pw")
                nc.gpsimd.normalize_recip(out_ap=pw[:], in_ap=e[:], denom_ap=rowsum[:])
                rinv = rowsum
                nc.scalar.dma_start(out=d_aw.ap()[b0], in_=pw[0:NQ, :])
                nc.scalar.dma_start(out=d_aw.ap()[b0 + 1], in_=pw[32 : 32 + NQ, :])
                ev = work.tile([64, N], F32, tag="ev")
                nume = small.tile([64, 1], F32, tag="nume")
                nc.vector.scalar_tensor_tensor(
                    out=ev[:], in0=e[:], scalar=1.0, in1=vb[:],
                    op0=Alu.mult, op1=Alu.mult, accum_out=nume[:],
                )
                l1 = small.tile([64, 1], F32, tag="l1")
                nc.vector.scalar_tensor_tensor(
                    out=l1[:], in0=nume[:], scalar=rinv[:],
                    in1=cc_sb[:, 1:2], op0=Alu.mult, op1=Alu.add,
                )
                nc.vector.scalar_tensor_tensor(
                    out=logits_all[:, j : j + 1], in0=l1[:],
                    scalar=qm_sb[:, j : j + 1], in1=cc_sb[:, 2:3],
                    op0=Alu.mult, op1=Alu.add,
                )

        nc.sync.dma_start(out=d_lg.ap(), in_=logits_all[:])

    nc.finalize()
    return nc


def _host_prep(latents, query_mask, query, ln_lat_w, ln_lat_b, ln_q_w, ln_q_b,
               Wq, bq, Wk, bk, Wv, bv, Wo, bo):
    f64 = np.float64
    pet = _pos_encoding_T(D, N)  # [D, N] float64

    q = query.astype(f64) * SCALE
    q_ln = _layernorm_np(q, ln_q_w.astype(f64), ln_q_b.astype(f64))
    qh = q_ln @ Wq.astype(f64).T + bq.astype(f64)          # [16, D]
    G = qh @ Wk.astype(f64)                                 # [16, D]
    w = ln_lat_w.astype(f64)
    b_ = ln_lat_b.astype(f64)
    gdev = (SCALE / math.sqrt(D)) * (G * w)                 # == G * w (H=1)
    sG = gdev.sum(-1)                                       # [16]
    gdev2 = gdev - sG[:, None] / D                          # fold mean-centering
    wv_eff = (Wo.astype(f64) @ Wv.astype(f64))[0]           # [D]
    wvdev = SCALE * (wv_eff * w)                            # [D]
    sv = float(wvdev.sum())
    wvdev2 = wvdev - sv / D
    c_v = float(wv_eff @ b_ + Wo.astype(f64)[0] @ bv.astype(f64))
    bo_val = float(np.asarray(bo).reshape(-1)[0])

    # lat' = latents + PE^T/scale, bf16, laid out [PB, 128, KC, N] per core
    latp = latents.astype(np.float32) + (pet / SCALE).astype(np.float32)[None]
    latp = latp.astype(ml_dtypes.bfloat16)                  # [B, D, N]
    latp = latp.reshape(B, KC, 128, N).transpose(0, 2, 1, 3)  # [B, 128, KC, N]
    latp = np.ascontiguousarray(latp)

    # lhsT [128, KC, 32]: cols 0..15 = gdev2^T, 16 = ones, 17 = wvdev2, rest 0
    lhsT = np.zeros((KC, 128, 32), np.float32)
    gdT = gdev2.T.reshape(KC, 128, NQ)                      # [KC, 128, 16]
    lhsT[:, :, 0:NQ] = gdT
    lhsT[:, :, NQ] = 1.0
    lhsT[:, :, NQ + 1] = wvdev2.reshape(KC, 128)
    lhsT = np.ascontiguousarray(
        lhsT.transpose(1, 0, 2)).astype(ml_dtypes.bfloat16)  # [128, KC, 32]

    uw = np.zeros((128, 2, 32), np.float32)
    uw[:, 0, 0] = 1.0
    uw[:, 1, 1] = 1.0
    uw = uw.astype(ml_dtypes.bfloat16)

    qm01 = (query_mask.astype(np.float32) != 0).astype(np.float32)  # [B, 16]

    cc = np.zeros((64, 4), np.float32)
    cc[:, 1] = c_v
    cc[:, 2] = bo_val
    cc[:, 3] = EPS

    in_maps = []
    for core in range(NCORES):
        bsl = slice(core * PB, (core + 1) * PB)
        qmc = qm01[bsl]                                     # [PB, 16]
        qm64 = np.zeros((64, NPAIR), np.float32)
        for j in range(NPAIR):
            qm64[0:16, j] = qmc[2 * j]
            qm64[32:48, j] = qmc[2 * j + 1]
        in_maps.append({
            "latp": latp[bsl],
            "lhsT": lhsT,
            "uw": uw,
            "qm": qm64,
            "cc": cc,
        })
    return in_maps


def kernel(latents, query_mask, query, ln_lat_w, ln_lat_b, ln_q_w, ln_q_b,
           Wq, bq, Wk, bk, Wv, bv, Wo, bo):
    args = [np.asarray(a) for a in (
        latents, query_mask, query, ln_lat_w, ln_lat_b, ln_q_w, ln_q_b,
        Wq, bq, Wk, bk, Wv, bv, Wo, bo)]
    in_maps = _host_prep(*args)
    nc = _build_kernel()
    res = run_bass_kernel_spmd(nc, in_maps, core_ids=list(range(NCORES)))
    aw = np.concatenate([r["aw"] for r in res.results], axis=0)   # [B, 16, N]
    lgs = []
    for r in res.results:
        lg64 = r["lg"]                                       # [64, NPAIR]
        lgc = np.empty((PB, NQ), np.float32)
        for j in range(NPAIR):
            lgc[2 * j] = lg64[0:16, j]
            lgc[2 * j + 1] = lg64[32:48, j]
        lgs.append(lgc)
    logits = np.concatenate(lgs, axis=0)[:, :, None].astype(np.float32)
    attn_weights = aw[:, None, :, :].astype(np.float32)
    return logits, attn_weights


# revision 39
# speedup vs baseline: 1.2153x; 1.0615x over previous
"""Self-contained Trainium2 Bass kernel for nn_AttentionPooling.

Contract: kernel(**inputs) takes FULL unsharded numpy inputs (as produced by
setup_inputs) and returns the FULL output tuple (logits [128,16,1],
attn_weights [128,1,16,512]), both float32.

Strategy
--------
Data-parallel over batch: 8 NeuronCores x 16 batch elements each, processed
in PAIRS to halve per-instruction overhead (a [64,512] vector op costs the
same as a [16,512] one — cost scales with the free dimension only).

Math restructuring (exact, validated vs the reference to ~3e-6 rel):
Because H == 1 and the query is batch-independent, the Q/K projections
collapse into one effective [16, 512] matrix G' = ((LN(query*scale) @ Wq.T
+ bq) @ Wk) * ln_lat_w, and the V/output projections collapse into a single
effective vector wvdev = scale * (Wo @ Wv) * ln_lat_w.  The latent-side
LayerNorm folds algebraically into per-column statistics (mean and rstd of
x = scale * lat') computed with extra matmul rows:

  lat'      = latents + PE^T / scale           (host, exact; then bf16)
  A2[q,n]   = sum_d gdev2[q,d] lat'[d,n]       (PE; gdev2 = G' - sG/512
                                                folds the mean-centering)
  slat[n]   = sum_d lat'[d,n]                  (PE, lhsT col 16 = ones)
  svraw2[n] = sum_d wvdev2[d] lat'[d,n]        (PE, col 17; wvdev2 centered)
  su[n]     = sum_d lat'[d,n]^2                (PE over u = lat'*lat')
  m~ = slat/512,  var = scale^2 (su/512 - m~^2),  r = 1/sqrt(var + eps)
  S[q,n]    = qm01[q] * r[n] * A2[q,n]
  attn_w    = softmax_n(S)      (no max-subtraction: |S| <~ 2 for this data)
  vWo[n]    = r[n] * svraw2[n]
  logits[q] = qm01[q] * (sum_n e[q,n] vWo[n] / sum_n e[q,n] + c_v) + bo

Row-constant score shifts are dropped (softmax-invariant), and the
query_mask path is exact: a zero mask row zeroes S (uniform softmax,
matching the reference's -1e9 fill) and routes logits to bo.

Per-pair PSUM layout (one bank, partitions):
  [ 0:32)  batch b0: A2 rows 0-15, slat 16, svraw2 17, zeros 18-31
  [32:64)  batch b1: same at +32
  [64:96)  su accumulators: row 64 = su(b0), row 65 = su(b1), rest zeros
Phase-2 ops run on the full [0:64) range with "junk lanes" masked by a
zero query-mask column — same instruction cost, half the instruction count.
"""

import math
from contextlib import ExitStack

import numpy as np
import ml_dtypes

import concourse.bass as bass
import concourse.mybir as mybir
import concourse.tile as tile
from concourse import bacc
from concourse.bass_utils import run_bass_kernel_spmd

F32 = mybir.dt.float32
BF16 = mybir.dt.bfloat16
Alu = mybir.AluOpType
Act = mybir.ActivationFunctionType

D = 512          # d_model
N = 512          # sequence length
NQ = 16          # number of queries
B = 128          # batch
NCORES = 8
PB = B // NCORES  # batches per core = 16
NPAIR = PB // 2
SCALE = math.sqrt(D)
EPS = 1e-5
KC = 4            # contraction chunks (512 / 128)
GROUP = 16       # batches per stats group (single group: dense PE burst)
PAIRS_PER_GROUP = GROUP // 2


def _pos_encoding_T(d_model: int, seq_len: int) -> np.ndarray:
    """Reversed sinusoidal PE, transposed to [D, N] (float64)."""
    pos = np.arange(seq_len, dtype=np.float64)[:, None]
    i = np.arange(0, d_model, 2, dtype=np.float64)
    ang = pos * np.exp(i * (-(math.log(10000.0) / d_model)))  # [N, d/2]
    pe = np.stack([np.sin(ang), np.cos(ang)], axis=-1).reshape(seq_len, d_model)
    return pe[::-1].T.copy()  # [D, N]


def _layernorm_np(x, w, b, eps=1e-5):
    m = x.mean(-1, keepdims=True)
    v = ((x - m) ** 2).mean(-1, keepdims=True)
    return (x - m) / np.sqrt(v + eps) * w + b


def _build_kernel():
    """Build the per-core Bass program (identical across cores)."""
    nc = bacc.Bacc("TRN2", target_bir_lowering=False, debug=False)

    d_latp = nc.dram_tensor("latp", [PB, 128, KC, N], BF16, kind="ExternalInput")
    d_lhsT = nc.dram_tensor("lhsT", [128, KC, 32], BF16, kind="ExternalInput")
    # u-matmul weights: [:, 0, :] has ones in col 0 (b0), [:, 1, :] ones in
    # col 1 (b1); all other cols zero.
    d_uw = nc.dram_tensor("uw", [128, 2, 32], BF16, kind="ExternalInput")
    d_qm = nc.dram_tensor("qm", [64, NPAIR], F32, kind="ExternalInput")
    # cc columns: 0 = unused, 1 = c_v, 2 = bo, 3 = eps
    d_cc = nc.dram_tensor("cc", [64, 4], F32, kind="ExternalInput")
    d_aw = nc.dram_tensor("aw", [PB, NQ, N], F32, kind="ExternalOutput")
    d_lg = nc.dram_tensor("lg", [64, NPAIR], F32, kind="ExternalOutput")

    with tile.TileContext(nc) as tc, ExitStack() as ctx:
        const = ctx.enter_context(tc.tile_pool(name="const", bufs=1))
        gstats = ctx.enter_context(tc.tile_pool(name="gstats", bufs=2))
        latp = ctx.enter_context(tc.tile_pool(name="latp", bufs=5))
        up = ctx.enter_context(tc.tile_pool(name="up", bufs=3))
        psum = ctx.enter_context(tc.tile_pool(name="psum", bufs=6, space="PSUM"))
        apool = ctx.enter_context(tc.tile_pool(name="apool", bufs=PAIRS_PER_GROUP + 2))
        work = ctx.enter_context(tc.tile_pool(name="work", bufs=4))
        small = ctx.enter_context(tc.tile_pool(name="small", bufs=4))
        dram = ctx.enter_context(tc.tile_pool(name="dram", bufs=1, space="DRAM"))
        r_dram = dram.tile([PB, N], F32, tag="r_dram")
        v_dram = dram.tile([PB, N], F32, tag="v_dram")

        lhsT_sb = const.tile([128, KC, 32], BF16, tag="lhsT")
        uw_sb = const.tile([128, 2, 32], BF16, tag="uw")
        qm_sb = const.tile([64, NPAIR], F32, tag="qm")
        cc_sb = const.tile([64, 4], F32, tag="cc")
        nc.sync.dma_start(out=lhsT_sb[:], in_=d_lhsT.ap())
        nc.sync.dma_start(out=uw_sb[:], in_=d_uw.ap())
        nc.sync.dma_start(out=qm_sb[:], in_=d_qm.ap())
        nc.sync.dma_start(out=cc_sb[:], in_=d_cc.ap())
        logits_all = const.tile([64, NPAIR], F32, tag="lg")

        c_inv_d = 1.0 / D
        c_var = (SCALE * SCALE) / D

        for g0 in range(0, PB, GROUP):
            # Per-group stats tiles (compute ops need 32-aligned partition
            # bases, so stacked rows live in group-local tiles at base 0).
            rows_g = gstats.tile([GROUP, 3, N], F32, tag="rows")
            slat_g = rows_g[:, 0, :]
            svraw_g = rows_g[:, 1, :]
            su_g = rows_g[:, 2, :]
            m_g = gstats.tile([GROUP, N], F32, tag="m")
            m2s_g = gstats.tile([GROUP, N], F32, tag="m2s")
            var_g = gstats.tile([GROUP, N], F32, tag="var")
            rstd_g = gstats.tile([GROUP, N], F32, tag="rstd")
            r_g = gstats.tile([GROUP, N], F32, tag="r")
            vwo_g = gstats.tile([GROUP, N], F32, tag="vwo")
            a_tiles = {}
            for j in range(g0 // 2, g0 // 2 + PAIRS_PER_GROUP):
                b0 = 2 * j
                i0 = b0 - g0  # row offset within the group tiles
                lat_p = latp.tile([128, 2, KC, N], BF16, tag="lat")
                nc.sync.dma_start(
                    out=lat_p[:],
                    in_=d_latp.ap()[b0 : b0 + 2].rearrange("b p c n -> p b c n"),
                )
                u_p = up.tile([128, 2, KC, N], BF16, tag="u")
                for i in range(2):
                    nc.vector.tensor_mul(
                        u_p[:, i].rearrange("p c n -> p (c n)"),
                        lat_p[:, i].rearrange("p c n -> p (c n)"),
                        lat_p[:, i].rearrange("p c n -> p (c n)"),
                    )
                ps = psum.tile([128, N], F32, tag="ps")
                for i in range(2):
                    for c in range(KC):
                        nc.tensor.matmul(
                            ps[32 * i : 32 * i + 32, :], lhsT_sb[:, c, :],
                            lat_p[:, i, c, :],
                            start=(c == 0), stop=(c == KC - 1),
                        )
                for i in range(2):
                    for c in range(KC):
                        nc.tensor.matmul(
                            ps[64:96, :], uw_sb[:, i, :], u_p[:, i, c, :],
                            start=(i == 0 and c == 0),
                            stop=(i == 1 and c == KC - 1),
                        )
                a_p = apool.tile([66, N], F32, tag="A")
                nc.scalar.copy(a_p[:], ps[0:66, :])
                # Gather stats rows (slat @16/48, svraw @17/49, su @64/65)
                # into per-batch slots of rows_g: [batch, {slat,svraw,su}, N].
                nc.scalar.dma_start(
                    out=rows_g[i0 : i0 + 1, 0:2, :], in_=a_p[16:18, :]
                )
                nc.scalar.dma_start(
                    out=rows_g[i0 + 1 : i0 + 2, 0:2, :], in_=a_p[48:50, :]
                )
                nc.scalar.dma_start(
                    out=rows_g[i0 : i0 + 2, 2:3, :], in_=a_p[64:66, :]
                )
                a_tiles[j] = a_p

            # Batched LN statistics for the group (per-column mean / rstd).
            nc.vector.scalar_tensor_tensor(
                out=m2s_g[:], in0=slat_g, scalar=(SCALE * SCALE) / (D * D),
                in1=slat_g, op0=Alu.mult, op1=Alu.mult,
            )
            nc.vector.scalar_tensor_tensor(
                out=var_g[:], in0=su_g, scalar=c_var,
                in1=m2s_g[:], op0=Alu.mult, op1=Alu.subtract,
            )
            nc.scalar.activation(
                out=rstd_g[:], in_=var_g[:], func=Act.Sqrt,
                bias=cc_sb[0:GROUP, 3:4], scale=1.0,
            )
            nc.vector.reciprocal_approx_fast(out=r_g[:], in_=rstd_g[:])
            nc.vector.tensor_mul(vwo_g[:], svraw_g, r_g[:])
            gsl = slice(g0, g0 + GROUP)
            nc.sync.dma_start(out=r_dram[gsl, :], in_=r_g[:])
            nc.sync.dma_start(out=v_dram[gsl, :], in_=vwo_g[:])

            for j in range(g0 // 2, g0 // 2 + PAIRS_PER_GROUP):
                b0 = 2 * j
                a_p = a_tiles[j]
                rb = work.tile([64, N], F32, tag="rb")
                vb = work.tile([64, N], F32, tag="vb")
                nc.gpsimd.dma_start(
                    out=rb[:], in_=r_dram[b0 : b0 + 2, :].rearrange("b (x n) -> b x n", x=1).broadcast_to((2, 32, N))
                )
                nc.gpsimd.dma_start(
                    out=vb[:], in_=v_dram[b0 : b0 + 2, :].rearrange("b (x n) -> b x n", x=1).broadcast_to((2, 32, N))
                )
                s2 = work.tile([64, N], F32, tag="s2")
                nc.vector.scalar_tensor_tensor(
                    out=s2[:], in0=a_p[0:64, :], scalar=qm_sb[:, j : j + 1],
                    in1=rb[:], op0=Alu.mult, op1=Alu.mult,
                )
                e = work.tile([64, N], F32, tag="e")
                rowsum = small.tile([64, 1], F32, tag="rowsum")
                nc.scalar.activation(
                    out=e[:], in_=s2[:], func=Act.Exp, accum_out=rowsum[:]
                )
                rinv = small.tile([64, 1], F32, tag="rinv")
                nc.vector.reciprocal(out=rinv[:], in_=rowsum[:])
                pw = work.tile([64, N], F32, tag="pw")
                nc.scalar.activation(
                    out=pw[:], in_=e[:], func=Act.Copy, scale=rinv[:]
                )
                nc.scalar.dma_start(out=d_aw.ap()[b0], in_=pw[0:NQ, :])
                nc.scalar.dma_start(out=d_aw.ap()[b0 + 1], in_=pw[32 : 32 + NQ, :])
                ev = work.tile([64, N], F32, tag="ev")
                nume = small.tile([64, 1], F32, tag="nume")
                nc.vector.scalar_tensor_tensor(
                    out=ev[:], in0=e[:], scalar=1.0, in1=vb[:],
                    op0=Alu.mult, op1=Alu.mult, accum_out=nume[:],
                )
                l1 = small.tile([64, 1], F32, tag="l1")
                nc.vector.scalar_tensor_tensor(
                    out=l1[:], in0=nume[:], scalar=rinv[:],
                    in1=cc_sb[:, 1:2], op0=Alu.mult, op1=Alu.add,
                )
                nc.vector.scalar_tensor_tensor(
                    out=logits_all[:, j : j + 1], in0=l1[:],
                    scalar=qm_sb[:, j : j + 1], in1=cc_sb[:, 2:3],
                    op0=Alu.mult, op1=Alu.add,
                )

        nc.sync.dma_start(out=d_lg.ap(), in_=logits_all[:])

    nc.finalize()
    return nc


def _host_prep(latents, query_mask, query, ln_lat_w, ln_lat_b, ln_q_w, ln_q_b,
               Wq, bq, Wk, bk, Wv, bv, Wo, bo):
    f64 = np.float64
    pet = _pos_encoding_T(D, N)  # [D, N] float64

    q = query.astype(f64) * SCALE
    q_ln = _layernorm_np(q, ln_q_w.astype(f64), ln_q_b.astype(f64))
    qh = q_ln @ Wq.astype(f64).T + bq.astype(f64)          # [16, D]
    G = qh @ Wk.astype(f64)                                 # [16, D]
    w = ln_lat_w.astype(f64)
    b_ = ln_lat_b.astype(f64)
    gdev = (SCALE / math.sqrt(D)) * (G * w)                 # == G * w (H=1)
    sG = gdev.sum(-1)                                       # [16]
    gdev2 = gdev - sG[:, None] / D                          # fold mean-centering
    wv_eff = (Wo.astype(f64) @ Wv.astype(f64))[0]           # [D]
    wvdev = SCALE * (wv_eff * w)                            # [D]
    sv = float(wvdev.sum())
    wvdev2 = wvdev - sv / D
    c_v = float(wv_eff @ b_ + Wo.astype(f64)[0] @ bv.astype(f64))
    bo_val = float(np.asarray(bo).reshape(-1)[0])

    # lat' = latents + PE^T/scale, bf16, laid out [PB, 128, KC, N] per core
    latp = latents.astype(np.float32) + (pet / SCALE).astype(np.float32)[None]
    latp = latp.astype(ml_dtypes.bfloat16)                  # [B, D, N]
    latp = latp.reshape(B, KC, 128, N).transpose(0, 2, 1, 3)  # [B, 128, KC, N]
    latp = np.ascontiguousarray(latp)

    # lhsT [128, KC, 32]: cols 0..15 = gdev2^T, 16 = ones, 17 = wvdev2, rest 0
    lhsT = np.zeros((KC, 128, 32), np.float32)
    gdT = gdev2.T.reshape(KC, 128, NQ)                      # [KC, 128, 16]
    lhsT[:, :, 0:NQ] = gdT
    lhsT[:, :, NQ] = 1.0
    lhsT[:, :, NQ + 1] = wvdev2.reshape(KC, 128)
    lhsT = np.ascontiguousarray(
        lhsT.transpose(1, 0, 2)).astype(ml_dtypes.bfloat16)  # [128, KC, 32]

    uw = np.zeros((128, 2, 32), np.float32)
    uw[:, 0, 0] = 1.0
    uw[:, 1, 1] = 1.0
    uw = uw.astype(ml_dtypes.bfloat16)

    qm01 = (query_mask.astype(np.float32) != 0).astype(np.float32)  # [B, 16]

    cc = np.zeros((64, 4), np.float32)
    cc[:, 1] = c_v
    cc[:, 2] = bo_val
    cc[:, 3] = EPS

    in_maps = []
    for core in range(NCORES):
        bsl = slice(core * PB, (core + 1) * PB)
        qmc = qm01[bsl]                                     # [PB, 16]
        qm64 = np.zeros((64, NPAIR), np.float32)
        for j in range(NPAIR):
            qm64[0:16, j] = qmc[2 * j]
            qm64[32:48, j] = qmc[2 * j + 1]
        in_maps.append({
            "latp": latp[bsl],
            "lhsT": lhsT,
            "uw": uw,
            "qm": qm64,
            "cc": cc,
        })
    return in_maps


def kernel(latents, query_mask, query, ln_lat_w, ln_lat_b, ln_q_w, ln_q_b,
           Wq, bq, Wk, bk, Wv, bv, Wo, bo):
    args = [np.asarray(a) for a in (
        latents, query_mask, query, ln_lat_w, ln_lat_b, ln_q_w, ln_q_b,
        Wq, bq, Wk, bk, Wv, bv, Wo, bo)]
    in_maps = _host_prep(*args)
    nc = _build_kernel()
    res = run_bass_kernel_spmd(nc, in_maps, core_ids=list(range(NCORES)))
    aw = np.concatenate([r["aw"] for r in res.results], axis=0)   # [B, 16, N]
    lgs = []
    for r in res.results:
        lg64 = r["lg"]                                       # [64, NPAIR]
        lgc = np.empty((PB, NQ), np.float32)
        for j in range(NPAIR):
            lgc[2 * j] = lg64[0:16, j]
            lgc[2 * j + 1] = lg64[32:48, j]
        lgs.append(lgc)
    logits = np.concatenate(lgs, axis=0)[:, :, None].astype(np.float32)
    attn_weights = aw[:, None, :, :].astype(np.float32)
    return logits, attn_weights
